# revision 5
# baseline (speedup 1.0000x reference)
"""Bahdanau (MLP) attention kernel for Trainium2, data-parallel over 8 NeuronCores.

Math per batch element b (one core each):
    qh[h,q] = sum_d Wq[h,d] query[q,d] + bq[h]          (PE + bias add)
    mh[h,m] = sum_d Wm[h,d] memory[m,d]                 (PE)
    t_q[h,m] = tanh(mh[h,m] + qh[h,q])                  (ACT, per-partition bias)
    attn[q,m] = sum_h v[h] t_q[h,m]                     (PE, v as stationary [128,1])
    attn += penalty (mask ? -1e30 : 0)                  (DVE)
    weights = softmax_m(attn)                           (DVE reduce + ACT exp)
    wm[q,d] = sum_m weights[q,m] memory[m,d]            (PE, weights transposed via PE)

Layout choice: H=128 lives in partitions for the tanh stage so the q-dependent
shift is a free per-partition bias on the ACT instruction.
"""

import sys

import numpy as np

sys.path.insert(0, "/opt/trn_rl_repo")

B, QLEN, MLEN = 8, 256, 1024
QS, MS, HID = 256, 256, 128
NCORES = 8
P = 128
NEG = -1.0e30

_compiled = {"nc": None}


def _build_bass():
    import concourse.bass as bass
    import concourse.tile as tile
    from concourse import bacc, mybir
    from concourse.masks import make_identity

    f32 = mybir.dt.float32
    bf16 = mybir.dt.bfloat16
    AF = mybir.ActivationFunctionType
    AX = mybir.AxisListType

    nc = bacc.Bacc("TRN2", target_bir_lowering=False, debug=False, num_devices=NCORES)

    queryT = nc.dram_tensor("queryT", [QS, QLEN], f32, kind="ExternalInput").ap()
    memT = nc.dram_tensor("memT", [MS, MLEN], f32, kind="ExternalInput").ap()
    mem = nc.dram_tensor("mem", [MLEN, MS], f32, kind="ExternalInput").ap()
    WqT = nc.dram_tensor("WqT", [QS, HID], f32, kind="ExternalInput").ap()
    WmT = nc.dram_tensor("WmT", [MS, HID], f32, kind="ExternalInput").ap()
    bq = nc.dram_tensor("bq", [HID, 1], f32, kind="ExternalInput").ap()
    v = nc.dram_tensor("v", [HID, 1], f32, kind="ExternalInput").ap()
    pen = nc.dram_tensor("pen", [1, MLEN], f32, kind="ExternalInput").ap()
    weights = nc.dram_tensor("weights", [QLEN, MLEN], f32, kind="ExternalOutput").ap()
    wm = nc.dram_tensor("wm", [QLEN, MS], f32, kind="ExternalOutput").ap()

    with tile.TileContext(nc) as tc:
        with (
            tc.tile_pool(name="singles", bufs=1) as singles,
            tc.tile_pool(name="tpool", bufs=3) as tpool,
            tc.tile_pool(name="soft", bufs=2) as soft,
            tc.tile_pool(name="pattn", bufs=1, space="PSUM") as pattn,
            tc.tile_pool(name="pmm", bufs=2, space="PSUM") as pmm,
        ):
            # ---- load inputs ------------------------------------------------
            memT_sb = singles.tile([P, 2, MLEN], f32)
            memT_r = memT.rearrange("(c p) m -> p c m", p=P)
            for c in range(2):
                for h in range(2):
                    sl = slice(h * 512, (h + 1) * 512)
                    nc.sync.dma_start(out=memT_sb[:, c, sl], in_=memT_r[:, c, sl])
            WmT_sb = singles.tile([P, 2, HID], f32)
            nc.sync.dma_start(out=WmT_sb, in_=WmT.rearrange("(c p) h -> p c h", p=P))
            WqT_sb = singles.tile([P, 2, HID], f32)
            nc.sync.dma_start(out=WqT_sb, in_=WqT.rearrange("(c p) h -> p c h", p=P))
            qT_sb = singles.tile([P, 2, QLEN], f32)
            nc.sync.dma_start(out=qT_sb, in_=queryT.rearrange("(c p) q -> p c q", p=P))
            bq_sb = singles.tile([P, 1], f32)
            nc.sync.dma_start(out=bq_sb, in_=bq)
            v32_sb = singles.tile([P, 1], f32)
            nc.sync.dma_start(out=v32_sb, in_=v)
            v_sb = singles.tile([P, 1], bf16)
            nc.vector.tensor_copy(out=v_sb, in_=v32_sb)

            mem_sb = singles.tile([P, 8, MS], f32)
            mem_r = mem.rearrange("(c p) d -> p c d", p=P)
            for c in range(8):
                nc.sync.dma_start(out=mem_sb[:, c, :], in_=mem_r[:, c, :])
            # mask penalty, broadcast the [1, MLEN] row to all 128 partitions
            pen_bc = singles.tile([P, MLEN], f32)
            pen_bcast_ap = bass.AP(
                tensor=pen.tensor, offset=pen.offset, ap=[[0, P]] + [pen.ap[1]]
            )
            nc.sync.dma_start(out=pen_bc, in_=pen_bcast_ap)

            ident = singles.tile([P, P], f32)
            make_identity(nc, ident)

            # ---- qh, mh -----------------------------------------------------
            qh_ps = pmm.tile([P, QLEN], f32, tag="mm")
            for c in range(2):
                nc.tensor.matmul(
                    out=qh_ps,
                    lhsT=WqT_sb[:, c, :],
                    rhs=qT_sb[:, c, :],
                    start=(c == 0),
                    stop=(c == 1),
                )
            qh_sb = singles.tile([P, QLEN], f32)
            nc.vector.tensor_scalar_add(qh_sb, qh_ps, bq_sb)

            mh_sb = singles.tile([P, MLEN], f32)
            for half in range(2):
                sl = slice(half * 512, (half + 1) * 512)
                mh_ps = pmm.tile([P, 512], f32, tag="mm")
                for c in range(2):
                    nc.tensor.matmul(
                        out=mh_ps,
                        lhsT=WmT_sb[:, c, :],
                        rhs=memT_sb[:, c, sl],
                        start=(c == 0),
                        stop=(c == 1),
                    )
                nc.vector.tensor_copy(out=mh_sb[:, sl], in_=mh_ps)

            # ---- hot loop: tanh + contraction with v ------------------------
            # attnT[m, q] accumulated in PSUM: 4 tiles, each holding 2 m-chunks
            attnT_ps = [
                pattn.tile([P, 2, QLEN], f32, tag=f"attnT{j}", name=f"attnT{j}")
                for j in range(4)
            ]
            for q in range(QLEN):
                t_sb = tpool.tile([P, MLEN], bf16, tag="t", name="t_sb")
                nc.scalar.activation(
                    out=t_sb,
                    in_=mh_sb,
                    func=AF.Tanh,
                    bias=qh_sb[:, q : q + 1],
                    scale=1.0,
                )
                for mc in range(8):
                    nc.tensor.matmul(
                        out=attnT_ps[mc // 2][:, mc % 2, q : q + 1],
                        lhsT=t_sb[:, mc * P : (mc + 1) * P],
                        rhs=v_sb,
                        start=True,
                        stop=True,
                    )

            # ---- attnT (PSUM) -> SBUF -> transpose to attn[q, m] ------------
            attnT_sb = singles.tile([P, 8, QLEN], f32)
            for j in range(4):
                nc.vector.tensor_copy(
                    out=attnT_sb[:, 2 * j : 2 * j + 2, :], in_=attnT_ps[j]
                )
            attn_sb = singles.tile([P, 2, MLEN], f32)
            for qb in range(2):
                for mc in range(8):
                    at_ps = pmm.tile([P, P], f32, tag="mm", name="at_ps")
                    nc.tensor.transpose(
                        out=at_ps,
                        in_=attnT_sb[:, mc, qb * P : (qb + 1) * P],
                        identity=ident,
                    )
                    nc.vector.tensor_copy(
                        out=attn_sb[:, qb, mc * P : (mc + 1) * P], in_=at_ps
                    )

            # ---- mask + softmax + outputs per q-block -----------------------
            w_sbs = []
            for qb in range(2):
                a_sb = soft.tile([P, MLEN], f32, tag="a", name="a_sb")
                nc.vector.tensor_add(a_sb, attn_sb[:, qb, :], pen_bc)
                mx = soft.tile([P, 1], f32, tag="mx", name="mx")
                nc.vector.reduce_max(out=mx, in_=a_sb, axis=AX.X)
                nmx = soft.tile([P, 1], f32, tag="nmx", name="nmx")
                nc.vector.tensor_scalar_mul(nmx, mx, -1.0)
                e_sb = soft.tile([P, MLEN], f32, tag="e", name="e_sb")
                ssum = soft.tile([P, 1], f32, tag="ssum", name="ssum")
                nc.scalar.activation(
                    out=e_sb,
                    in_=a_sb,
                    func=AF.Exp,
                    bias=nmx,
                    scale=1.0,
                    accum_out=ssum,
                )
                rs = soft.tile([P, 1], f32, tag="rs", name="rs")
                nc.vector.reciprocal(out=rs, in_=ssum)
                w_sb = singles.tile([P, MLEN], f32, tag=f"w{qb}", name=f"w{qb}")
                nc.vector.tensor_scalar_mul(w_sb, e_sb, rs)
                for h in range(2):
                    sl = slice(h * 512, (h + 1) * 512)
                    nc.sync.dma_start(out=weights[qb * P : (qb + 1) * P, sl], in_=w_sb[:, sl])
                w_sbs.append(w_sb)

            # ---- weighted memory: wm = weights @ memory ---------------------
            for qb in range(2):
                wT_sb = singles.tile([P, 8, P], f32, tag=f"wT{qb}", name=f"wT{qb}")
                for mc in range(8):
                    tp_ps = pmm.tile([P, P], f32, tag="mm", name="tp_ps")
                    nc.tensor.transpose(
                        out=tp_ps, in_=w_sbs[qb][:, mc * P : (mc + 1) * P], identity=ident
                    )
                    nc.vector.tensor_copy(out=wT_sb[:, mc, :], in_=tp_ps)
                out_ps = pmm.tile([P, MS], f32, tag="mm", name="out_ps")
                for mc in range(8):
                    nc.tensor.matmul(
                        out=out_ps,
                        lhsT=wT_sb[:, mc, :],
                        rhs=mem_sb[:, mc, :],
                        start=(mc == 0),
                        stop=(mc == 7),
                    )
                out_sb = soft.tile([P, MS], f32, tag="out", name="out_sb")
                nc.vector.tensor_copy(out=out_sb, in_=out_ps)
                nc.sync.dma_start(out=wm[qb * P : (qb + 1) * P, :], in_=out_sb)

    nc.compile()
    return nc


def _get_nc():
    if _compiled["nc"] is None:
        _compiled["nc"] = _build_bass()
    return _compiled["nc"]


def kernel(query, memory, Wq, bq, Wm, v, mask, _trace=False):
    from concourse.bass_utils import run_bass_kernel_spmd

    query = np.asarray(query, dtype=np.float32)
    memory = np.asarray(memory, dtype=np.float32)
    Wq = np.asarray(Wq, dtype=np.float32)
    bq = np.asarray(bq, dtype=np.float32)
    Wm = np.asarray(Wm, dtype=np.float32)
    v = np.asarray(v, dtype=np.float32)
    mask = np.asarray(mask)

    nc = _get_nc()

    WqT = np.ascontiguousarray(Wq.T)
    WmT = np.ascontiguousarray(Wm.T)
    bq_c = np.ascontiguousarray(bq.reshape(HID, 1))
    v_c = np.ascontiguousarray(v.reshape(HID, 1))

    in_maps = []
    for b in range(NCORES):
        in_maps.append(
            {
                "queryT": np.ascontiguousarray(query[b].T),
                "memT": np.ascontiguousarray(memory[b].T),
                "mem": np.ascontiguousarray(memory[b]),
                "WqT": WqT,
                "WmT": WmT,
                "bq": bq_c,
                "v": v_c,
                "pen": np.where(mask[b], np.float32(NEG), np.float32(0.0)).reshape(
                    1, MLEN
                ).astype(np.float32),
            }
        )

    res = run_bass_kernel_spmd(nc, in_maps, list(range(NCORES)), trace=_trace)
    results = res.results

    wm_full = np.stack([results[i]["wm"] for i in range(NCORES)])
    weights_full = np.stack([results[i]["weights"] for i in range(NCORES)])
    if _trace:
        kernel.last_exec_time_ns = res.exec_time_ns
        kernel.last_trace = res.instructions_and_trace
    return wm_full, weights_full


# revision 6
# speedup vs baseline: 1.6615x; 1.6615x over previous
"""Bahdanau (MLP) attention kernel for Trainium2, data-parallel over 8 NeuronCores.

Math per batch element b (one core each):
    qh[h,q] = sum_d Wq[h,d] query[q,d] + bq[h]          (PE)
    mh[h,m] = sum_d Wm[h,d] memory[m,d]                 (PE)
    t_q[h,m] = tanh(mh[h,m] + qh[h,q])                  (DVE pre-add + ACT tanh)
    attn[q,m] = sum_h v[h] t_q[h,m]                     (PE, t chunk stationary)
    weights = softmax_m(attn + penalty)                 (DVE reduce + ACT exp)
    wm[q,d] = sum_m weights[q,m] memory[m,d]            (PE, weights transposed via PE)

Key optimizations:
  - H=128 in partitions so the q-dependent shift is a per-partition scalar.
  - Masked memory positions produce exactly-0 softmax weights, so unmasked
    columns are gathered host-side (compaction) and results scattered back;
    the device only processes MC = ceil(max_unmasked/128)*128 columns.
  - The +qh add runs on the (otherwise idle) DVE at 2 elem/cycle, batching 8
    q's into one big ACT tanh to amortize per-instruction overhead.
  - The v-contraction uses bf16 (t, v) at 1 PE cycle/row; everything else fp32.
"""

import sys

import numpy as np

sys.path.insert(0, "/opt/trn_rl_repo")

B, QLEN, MLEN = 8, 256, 1024
QS, MS, HID = 256, 256, 128
NCORES = 8
P = 128
NEG = -1.0e30
GK = 8  # q's per tanh batch

_compiled = {}


def _build_bass(MC):
    import concourse.bass as bass
    import concourse.tile as tile
    from concourse import bacc, mybir

    f32 = mybir.dt.float32
    bf16 = mybir.dt.bfloat16
    AF = mybir.ActivationFunctionType
    AX = mybir.AxisListType

    n_mc = MC // P
    n_pair = (n_mc + 1) // 2

    nc = bacc.Bacc("TRN2", target_bir_lowering=False, debug=False, num_devices=NCORES)

    queryT = nc.dram_tensor("queryT", [QS, QLEN], f32, kind="ExternalInput").ap()
    memT = nc.dram_tensor("memT", [MS, MC], f32, kind="ExternalInput").ap()
    mem = nc.dram_tensor("mem", [MC, MS], f32, kind="ExternalInput").ap()
    WqT = nc.dram_tensor("WqT", [QS, HID], f32, kind="ExternalInput").ap()
    WmT = nc.dram_tensor("WmT", [MS, HID], f32, kind="ExternalInput").ap()
    bq = nc.dram_tensor("bq", [HID, 1], f32, kind="ExternalInput").ap()
    v = nc.dram_tensor("v", [HID, 1], f32, kind="ExternalInput").ap()
    pen = nc.dram_tensor("pen", [1, MC], f32, kind="ExternalInput").ap()
    identd = nc.dram_tensor("identd", [P, P], f32, kind="ExternalInput").ap()
    weights = nc.dram_tensor("weights", [QLEN, MC], f32, kind="ExternalOutput").ap()
    wm = nc.dram_tensor("wm", [QLEN, MS], f32, kind="ExternalOutput").ap()

    with tile.TileContext(nc) as tc:
        with (
            tc.tile_pool(name="singles", bufs=1) as singles,
            tc.tile_pool(name="upool", bufs=2) as upool,
            tc.tile_pool(name="tpool", bufs=2) as tpool,
            tc.tile_pool(name="soft", bufs=2) as soft,
            tc.tile_pool(name="pattn", bufs=1, space="PSUM") as pattn,
            tc.tile_pool(name="pmm", bufs=2, space="PSUM") as pmm,
        ):
            # ---- load inputs ------------------------------------------------
            memT_sb = singles.tile([P, 2, MC], f32)
            memT_r = memT.rearrange("(c p) m -> p c m", p=P)
            for c in range(2):
                for off in range(0, MC, 512):
                    w = min(512, MC - off)
                    sl = slice(off, off + w)
                    nc.sync.dma_start(out=memT_sb[:, c, sl], in_=memT_r[:, c, sl])
            WmT_sb = singles.tile([P, 2, HID], f32)
            nc.sync.dma_start(out=WmT_sb, in_=WmT.rearrange("(c p) h -> p c h", p=P))
            WqT_sb = singles.tile([P, 2, HID], f32)
            nc.sync.dma_start(out=WqT_sb, in_=WqT.rearrange("(c p) h -> p c h", p=P))
            qT_sb = singles.tile([P, 2, QLEN], f32)
            nc.sync.dma_start(out=qT_sb, in_=queryT.rearrange("(c p) q -> p c q", p=P))
            bq_sb = singles.tile([P, 1], f32)
            nc.sync.dma_start(out=bq_sb, in_=bq)
            v32_sb = singles.tile([P, 1], f32)
            nc.sync.dma_start(out=v32_sb, in_=v)
            v_sb = singles.tile([P, 1], bf16)
            nc.scalar.activation(out=v_sb, in_=v32_sb, func=AF.Identity)
            ident = singles.tile([P, P], f32)
            nc.sync.dma_start(out=ident, in_=identd)

            mem_sb = singles.tile([P, n_mc, MS], f32)
            mem_r = mem.rearrange("(c p) d -> p c d", p=P)
            for c in range(n_mc):
                nc.sync.dma_start(out=mem_sb[:, c, :], in_=mem_r[:, c, :])
            pen_bc = singles.tile([P, MC], f32)
            pen_bcast_ap = bass.AP(
                tensor=pen.tensor, offset=pen.offset, ap=[[0, P]] + [pen.ap[1]]
            )
            nc.sync.dma_start(out=pen_bc, in_=pen_bcast_ap)

            # ---- qh, mh -----------------------------------------------------
            qh_ps = pmm.tile([P, QLEN], f32, tag="mm")
            for c in range(2):
                nc.tensor.matmul(
                    out=qh_ps,
                    lhsT=WqT_sb[:, c, :],
                    rhs=qT_sb[:, c, :],
                    start=(c == 0),
                    stop=(c == 1),
                )
            qh_sb = singles.tile([P, QLEN], f32)
            nc.vector.tensor_scalar_add(qh_sb, qh_ps, bq_sb)

            mh_sb = singles.tile([P, MC], f32)
            for off in range(0, MC, 512):
                w = min(512, MC - off)
                sl = slice(off, off + w)
                mh_ps = pmm.tile([P, 512], f32, tag="mm", name="mh_ps")
                for c in range(2):
                    nc.tensor.matmul(
                        out=mh_ps[:, :w],
                        lhsT=WmT_sb[:, c, :],
                        rhs=memT_sb[:, c, sl],
                        start=(c == 0),
                        stop=(c == 1),
                    )
                nc.vector.tensor_copy(out=mh_sb[:, sl], in_=mh_ps[:, :w])

            # ---- hot loop ---------------------------------------------------
            # attnT[m, q] in PSUM, split by (m-chunk pair, q-block) so the
            # q<128 epilogue can run while q>=128 is still being produced.
            attn_ps = {}
            for j in range(n_pair):
                for qb in range(2):
                    attn_ps[(j, qb)] = pattn.tile(
                        [P, 2, P], f32, tag=f"attnT{j}_{qb}", name=f"attnT{j}_{qb}"
                    )

            def epilogue(qb):
                # attnT psum -> sbuf
                attnT_sb = soft.tile(
                    [P, n_mc, P], f32, tag=f"attnT_sb{qb}", name="attnT_sb"
                )
                for j in range(n_pair):
                    w = min(2, n_mc - 2 * j)
                    nc.vector.tensor_copy(
                        out=attnT_sb[:, 2 * j : 2 * j + w, :],
                        in_=attn_ps[(j, qb)][:, :w, :],
                    )
                # transpose to attn[q, m]
                a_sb = soft.tile([P, MC], f32, tag=f"a{qb}", name="a_sb")
                for mc in range(n_mc):
                    at_ps = pmm.tile([P, P], f32, tag="mm", name="at_ps")
                    nc.tensor.transpose(
                        out=at_ps, in_=attnT_sb[:, mc, :], identity=ident
                    )
                    nc.vector.tensor_copy(
                        out=a_sb[:, mc * P : (mc + 1) * P], in_=at_ps
                    )
                # mask + softmax
                am_sb = soft.tile([P, MC], f32, tag=f"am{qb}", name="am_sb")
                nc.vector.tensor_add(am_sb, a_sb, pen_bc)
                mx = soft.tile([P, 1], f32, tag="mx", name="mx")
                nc.vector.reduce_max(out=mx, in_=am_sb, axis=AX.X)
                nmx = soft.tile([P, 1], f32, tag="nmx", name="nmx")
                nc.vector.tensor_scalar_mul(nmx, mx, -1.0)
                e_sb = soft.tile([P, MC], f32, tag=f"e{qb}", name="e_sb")
                ssum = soft.tile([P, 1], f32, tag="ssum", name="ssum")
                nc.scalar.activation(
                    out=e_sb, in_=am_sb, func=AF.Exp, bias=nmx, scale=1.0,
                    accum_out=ssum,
                )
                rs = soft.tile([P, 1], f32, tag="rs", name="rs")
                nc.vector.reciprocal(out=rs, in_=ssum)
                w_sb = singles.tile([P, MC], f32, tag=f"w{qb}", name=f"w{qb}")
                nc.vector.tensor_scalar_mul(w_sb, e_sb, rs)
                for off in range(0, MC, 512):
                    w = min(512, MC - off)
                    sl = slice(off, off + w)
                    nc.sync.dma_start(
                        out=weights[qb * P : (qb + 1) * P, sl], in_=w_sb[:, sl]
                    )
                # wm = weights @ memory
                wT_sb = soft.tile([P, n_mc, P], f32, tag=f"wT{qb}", name="wT_sb")
                for mc in range(n_mc):
                    tp_ps = pmm.tile([P, P], f32, tag="mm", name="tp_ps")
                    nc.tensor.transpose(
                        out=tp_ps, in_=w_sb[:, mc * P : (mc + 1) * P], identity=ident
                    )
                    nc.vector.tensor_copy(out=wT_sb[:, mc, :], in_=tp_ps)
                out_ps = pmm.tile([P, MS], f32, tag="mm", name="out_ps")
                for mc in range(n_mc):
                    nc.tensor.matmul(
                        out=out_ps,
                        lhsT=wT_sb[:, mc, :],
                        rhs=mem_sb[:, mc, :],
                        start=(mc == 0),
                        stop=(mc == n_mc - 1),
                    )
                out_sb = soft.tile([P, MS], f32, tag=f"out{qb}", name="out_sb")
                nc.vector.tensor_copy(out=out_sb, in_=out_ps)
                nc.sync.dma_start(out=wm[qb * P : (qb + 1) * P, :], in_=out_sb)

            n_groups = QLEN // GK
            for g in range(n_groups):
                u_sb = upool.tile([P, GK, MC], f32, tag="u", name="u_sb")
                for j in range(GK):
                    q = g * GK + j
                    nc.vector.tensor_scalar_add(
                        u_sb[:, j, :], mh_sb, qh_sb[:, q : q + 1]
                    )
                t_sb = tpool.tile([P, GK, MC], bf16, tag="t", name="t_sb")
                nc.scalar.activation(out=t_sb, in_=u_sb, func=AF.Tanh)
                for j in range(GK):
                    q = g * GK + j
                    qb, qi = divmod(q, P)
                    for mc in range(n_mc):
                        nc.tensor.matmul(
                            out=attn_ps[(mc // 2, qb)][:, mc % 2, qi : qi + 1],
                            lhsT=t_sb[:, j, mc * P : (mc + 1) * P],
                            rhs=v_sb,
                            start=True,
                            stop=True,
                        )
                if g * GK + GK - 1 == P - 1:
                    epilogue(0)
            epilogue(1)

    nc.compile()
    return nc


def _get_nc(MC):
    if MC not in _compiled:
        _compiled[MC] = _build_bass(MC)
    return _compiled[MC]


def kernel(query, memory, Wq, bq, Wm, v, mask, _trace=False):
    from concourse.bass_utils import run_bass_kernel_spmd

    query = np.asarray(query, dtype=np.float32)
    memory = np.asarray(memory, dtype=np.float32)
    Wq = np.asarray(Wq, dtype=np.float32)
    bq = np.asarray(bq, dtype=np.float32)
    Wm = np.asarray(Wm, dtype=np.float32)
    v = np.asarray(v, dtype=np.float32)
    mask = np.asarray(mask).astype(bool)

    idxs = [np.nonzero(~mask[b])[0] for b in range(NCORES)]
    cnts = [len(ix) for ix in idxs]
    MC = max(P, ((max(cnts) + P - 1) // P) * P)
    MC = min(MC, MLEN)

    nc = _get_nc(MC)

    WqT = np.ascontiguousarray(Wq.T)
    WmT = np.ascontiguousarray(Wm.T)
    bq_c = np.ascontiguousarray(bq.reshape(HID, 1))
    v_c = np.ascontiguousarray(v.reshape(HID, 1))
    ident = np.eye(P, dtype=np.float32)

    in_maps = []
    for b in range(NCORES):
        ix, cnt = idxs[b], cnts[b]
        mem_c = np.zeros((MC, MS), dtype=np.float32)
        mem_c[:cnt] = memory[b][ix]
        pen_c = np.full((1, MC), NEG, dtype=np.float32)
        pen_c[0, :cnt] = 0.0
        in_maps.append(
            {
                "queryT": np.ascontiguousarray(query[b].T),
                "memT": np.ascontiguousarray(mem_c.T),
                "mem": mem_c,
                "WqT": WqT,
                "WmT": WmT,
                "bq": bq_c,
                "v": v_c,
                "pen": pen_c,
                "identd": ident,
            }
        )

    res = run_bass_kernel_spmd(nc, in_maps, list(range(NCORES)), trace=_trace)
    results = res.results

    wm_full = np.stack([results[i]["wm"] for i in range(NCORES)])
    weights_full = np.zeros((NCORES, QLEN, MLEN), dtype=np.float32)
    for b in range(NCORES):
        weights_full[b][:, idxs[b]] = results[b]["weights"][:, : cnts[b]]
    if _trace:
        kernel.last_exec_time_ns = res.exec_time_ns
        kernel.last_trace = res.instructions_and_trace
    return wm_full, weights_full
